# revision 3
# baseline (speedup 1.0000x reference)
"""GraphSAGE (3-layer, mean-aggr) on 8 Trainium2 NeuronCores.

Self-contained: host-side graph prep (numpy) + Bass kernel build + SPMD run.

Architecture per layer:
  P = h @ Wl per shard -> AllGather table T [Ng, 64]
  pass A: per (slot-region r, src-window w): dma_gather rows of T (int16 idx,
          windowed) -> dma_scatter_add into unique dst-grouped slots of a
          slot buffer (int16, windowed, collision-free)
  pass B: sequential reads of slot stream -> constant block-band matmuls on PE
          (aggregation, 1/deg folded into the constants; 64-node groups, PSUM
          partition starts 0/64) -> node-major M
  combine: h' = relu(M + S + b), S = h @ Wr kept from previous layer
  tail: per-128-node-tile PE transpose -> h_fm -> P/S for next layer
  L2: one-hot pooling matmul accumulated in PSUM -> AllReduce -> head.
"""
import numpy as np

import concourse.bass as bass
import concourse.mybir as mybir
import concourse.tile as tile
from concourse import bacc
from concourse.masks import make_identity

N_NODES = 250000
N_EDGES = 2000000
N_GRAPHS = 64
IN_DIM = 16
HID = 64
NCORE = 8
P = 128
WIN = 32768          # int16 index window (rows)
RUSE = 30720         # usable slots per region
RSCR = 2048          # scratch slots per region
MCALL = 16           # free-tiles per gather/scatter call (2048 rows)
CALLROWS = P * MCALL


def _wrap_idx(flat):
    """[n] -> [128, n//16] int16: stream i at [i%16, i//16], replicated x8."""
    n = flat.shape[0]
    base = flat.reshape(n // 16, 16).T.astype(np.int16)
    t = np.zeros((P, n // 16), np.int16)
    for g in range(8):
        t[g * 16:(g + 1) * 16] = base
    return t


def _prep(edge_index, batch):
    src_g = np.asarray(edge_index[0], np.int64)
    dst_g = np.asarray(edge_index[1], np.int64)
    batch = np.asarray(batch, np.int64)

    base = N_NODES // NCORE  # 31250
    core_of = np.minimum(dst_g // base, NCORE - 1)
    deg = np.bincount(dst_g, minlength=N_NODES)

    # per-core: nodes [c*base, (c+1)*base); class grouping by in-degree
    cores = []
    dmax = int(deg.max())
    census = np.zeros((NCORE, dmax + 1), np.int64)
    for c in range(NCORE):
        lo, hi = c * base, (c + 1) * base if c < NCORE - 1 else N_NODES
        nodes = np.arange(lo, hi)
        d = deg[lo:hi]
        order = np.argsort(d, kind="stable")
        cores.append((nodes[order], d[order]))
        for dd in range(dmax + 1):
            census[c, dd] = int((d == dd).sum())
    # equalize: per-class node count = max over cores, padded to multiple of 64
    ncls = np.zeros(dmax + 1, np.int64)
    for dd in range(dmax + 1):
        m = int(census[:, dd].max())
        if m > 0:
            ncls[dd] = ((m + 63) // 64) * 64
    n_store = int(ncls.sum())
    n_store = ((n_store + 127) // 128) * 128
    ncls[0] += n_store - int(ncls.sum())  # absorb rounding into class 0

    # storage layout: classes ascending d; within class: real nodes then fakes
    # slot stream: class 0 -> 1 slot/node (zero lhsT); class d -> d slots/node
    # groups of 64 nodes; per group ceil(64*max(d,1)/128) tiles of 128 slots
    cls_start = np.zeros(dmax + 2, np.int64)
    for dd in range(dmax + 1):
        cls_start[dd + 1] = cls_start[dd] + ncls[dd]

    # tile plan (shared across cores): list of (d, sub) per 128-slot tile and
    # group plan: per 64-group: (d, n_sub); slot position of (class,node,j)
    tiles = []          # (d, sub_idx)
    groups = []         # (d, n_sub)
    slot_of_group = []  # slot index of group start
    scur = 0
    for dd in range(dmax + 1):
        de = max(dd, 1)
        n_sub = (64 * de + 127) // 128
        for g in range(int(ncls[dd]) // 64):
            groups.append((dd, n_sub))
            slot_of_group.append(scur)
            for s in range(n_sub):
                tiles.append((dd, s))
            scur += n_sub * 128
    n_slots = scur
    nreg = (n_slots + RUSE - 1) // RUSE

    # per-core data
    per_core = {"gidx": [], "sidx": [], "perm": [], "bid": []}
    # slot id for (group, node_in_group j, edge k): group_slot + j*de + k
    # (pad slots j*de+k beyond group's 64*de... none: slots per group n_sub*128
    #  >= 64*de; layout: node j edges at [j*de, (j+1)*de) contiguous; pad tail)
    callplan = None
    for c in range(NCORE):
        nodes_sorted, d_sorted = cores[c]
        # storage position of each real node
        storage = np.full(n_store, -1, np.int64)   # storage -> orig node
        pos = {}
        cur = 0
        for dd in range(dmax + 1):
            sel = d_sorted == dd
            nn = nodes_sorted[sel]
            storage[cls_start[dd]:cls_start[dd] + len(nn)] = nn
            cur = 0
            for i, v in enumerate(nn):
                pos[v] = cls_start[dd] + i
        per_core["perm"].append(storage)
        bid = np.full(n_store, -1.0, np.float32)
        real = storage >= 0
        bid[real] = batch[storage[real]].astype(np.float32)
        per_core["bid"].append(bid)

        # edges of this core -> slots
        em = core_of == c
        es, ed = src_g[em], dst_g[em]
        # slot for edge: node storage pos -> group, j
        p_arr = np.array([pos[v] for v in ed], np.int64)
        dd_arr = deg[ed]
        grp = p_arr // 64
        j = p_arr % 64
        de_arr = np.maximum(dd_arr, 1)
        # rank of edge within its node (stable order)
        order = np.argsort(p_arr, kind="stable")
        es, ed, p_arr, grp, j, de_arr = (a[order] for a in (es, ed, p_arr, grp, j, de_arr))
        k = np.zeros(len(es), np.int64)
        if len(p_arr):
            brk = np.r_[True, p_arr[1:] != p_arr[:-1]]
            idx0 = np.flatnonzero(brk)
            k = np.arange(len(p_arr)) - np.repeat(idx0, np.diff(np.r_[idx0, len(p_arr)]))
        gstart = np.array(slot_of_group, np.int64)
        slot = gstart[grp] + j * de_arr + k
        # class-0 nodes: add one zero-slot edge each (src -> global row 0)
        z_pos = np.arange(cls_start[0], cls_start[1])
        z_grp = z_pos // 64
        z_slot = gstart[z_grp] + (z_pos % 64)
        # table row of src: core_of(src)*n_store + storage pos on that core
        # (need pos map of OTHER cores -> compute later; placeholder store orig)
        per_core["gidx"].append((es, slot, z_slot))

    # global storage position of every original node
    gpos = np.zeros(N_NODES, np.int64)
    for c in range(NCORE):
        st = per_core["perm"][c]
        real = st >= 0
        gpos[st[real]] = c * n_store + np.flatnonzero(real)

    # build call plans (identical structure across cores)
    # region r: slots [r*32768+0, ... r*32768+30720) hold stream chunk
    # stream slot s -> buffer address r*32768 + (s - r*RUSE)
    nwin = (NCORE * n_store + WIN - 1) // WIN
    cnt = np.zeros((NCORE, nreg, nwin), np.int64)
    edata = []
    for c in range(NCORE):
        es, slot, z_slot = per_core["gidx"][c]
        row = gpos[es]
        w = row // WIN
        r = slot // RUSE
        zrow = np.zeros(len(z_slot), np.int64)
        zw = zrow // WIN
        zr = z_slot // RUSE
        row = np.r_[row, zrow]
        w = np.r_[w, zw]
        slot_all = np.r_[slot, z_slot]
        r = np.r_[r, zr]
        key = r * nwin + w
        bc = np.bincount(key, minlength=nreg * nwin)
        cnt[c] = bc.reshape(nreg, nwin)
        order2 = np.argsort(key, kind="stable")
        edata.append((row[order2], w[order2], slot_all[order2], r[order2],
                      np.r_[0, np.cumsum(bc)]))
    ncall = np.zeros((nreg, nwin), np.int64)
    for rr in range(nreg):
        for ww in range(nwin):
            m = int(cnt[:, rr, ww].max())
            ncall[rr, ww] = (m + CALLROWS - 1) // CALLROWS

    GIDX, SIDX = [], []
    for c in range(NCORE):
        row, w, slot_all, r, csum = edata[c]
        gi_cols, si_cols = [], []
        for rr in range(nreg):
            for ww in range(nwin):
                need = int(ncall[rr, ww] * CALLROWS)
                a, b = csum[rr * nwin + ww], csum[rr * nwin + ww + 1]
                rws = row[a:b] - ww * WIN
                sls = slot_all[a:b] - rr * RUSE
                npad = need - len(rws)
                scr = RUSE + (np.arange(npad) % RSCR)
                rws = np.r_[rws, np.zeros(npad, np.int64)]
                sls = np.r_[sls, scr]
                for kk in range(int(ncall[rr, ww])):
                    gi_cols.append(_wrap_idx(rws[kk * CALLROWS:(kk + 1) * CALLROWS]))
                    si_cols.append(_wrap_idx(sls[kk * CALLROWS:(kk + 1) * CALLROWS]))
        GIDX.append(np.concatenate(gi_cols, 1))
        SIDX.append(np.concatenate(si_cols, 1))

    # lhsT constants: per (d, sub): [128, 64] with lhsT[s, jj] = 1/d if slot
    # sub*128+s belongs to node jj (slots j*de..j*de+de), 0 for class 0
    lhs = []
    lhs_of = {}
    for dd in range(dmax + 1):
        de = max(dd, 1)
        n_sub = (64 * de + 127) // 128
        for s in range(n_sub):
            m = np.zeros((P, 64), np.float32)
            if dd > 0:
                for srow in range(P):
                    gslot = s * 128 + srow
                    jj = gslot // de
                    if jj < 64:
                        m[srow, jj] = 1.0 / dd
            lhs_of[(dd, s)] = len(lhs)
            lhs.append(m)
    lhs = np.stack(lhs) if lhs else np.zeros((1, P, 64), np.float32)

    plan = dict(n_store=n_store, n_slots=n_slots, nreg=nreg, nwin=nwin,
                ncall=ncall, tiles=tiles, groups=groups, lhs_of=lhs_of,
                dmax=dmax)
    return plan, per_core, GIDX, SIDX, lhs, gpos


def _build(plan, gcols, scols, nlhs):
    nc = bacc.Bacc(None, target_bir_lowering=False)
    dt = mybir.dt.float32
    n_store = plan["n_store"]
    nreg, nwin, ncall = plan["nreg"], plan["nwin"], plan["ncall"]
    Ng = NCORE * n_store

    xT = nc.declare_dram_parameter("xT", [IN_DIM, n_store], dt, isOutput=False)
    gidx = nc.declare_dram_parameter("gidx", [P, gcols], mybir.dt.int16, isOutput=False)
    sidx = nc.declare_dram_parameter("sidx", [P, scols], mybir.dt.int16, isOutput=False)
    lhsc = nc.declare_dram_parameter("lhsc", [P, nlhs * 64], dt, isOutput=False)
    bidp = nc.declare_dram_parameter("bid", [n_store, 1], dt, isOutput=False)
    iotap = nc.declare_dram_parameter("iota", [P, 64], dt, isOutput=False)
    invc = nc.declare_dram_parameter("invc", [64, 1], dt, isOutput=False)
    Wp = {}
    for ell in range(3):
        kd = IN_DIM if ell == 0 else HID
        Wp[f"Wl{ell}"] = nc.declare_dram_parameter(f"Wl{ell}", [kd, HID], dt, isOutput=False)
        Wp[f"Wr{ell}"] = nc.declare_dram_parameter(f"Wr{ell}", [kd, HID], dt, isOutput=False)
        Wp[f"bl{ell}"] = nc.declare_dram_parameter(f"bl{ell}", [P, HID], dt, isOutput=False)
    Wp["Wh"] = nc.declare_dram_parameter("Wh", [HID, 1], dt, isOutput=False)
    Wp["bh"] = nc.declare_dram_parameter("bh", [64, 1], dt, isOutput=False)
    outp = nc.declare_dram_parameter("out", [64, 1], dt, isOutput=True)

    Pshard = nc.dram_tensor("Pshard", [n_store, HID], dt)
    Tglob = nc.dram_tensor("Tglob", [Ng, HID], dt, addr_space="Shared")
    slotbuf = nc.dram_tensor("slotbuf", [nreg * WIN, HID], dt)
    Scur = nc.dram_tensor("Scur", [n_store, HID], dt)
    Snxt = nc.dram_tensor("Snxt", [n_store, HID], dt)
    rawC = nc.dram_tensor("rawC", [n_store, HID], dt)
    arin = nc.dram_tensor("arin", [64, 64], dt)
    arout = nc.dram_tensor("arout", [64, 64], dt, addr_space="Shared")

    rep = [list(range(NCORE))]
    tiles, groups, lhs_of = plan["tiles"], plan["groups"], plan["lhs_of"]

    with tile.TileContext(nc) as tc:
        with (
            tc.tile_pool(name="const", bufs=1) as cp,
            tc.tile_pool(name="gb", bufs=8) as gp,
            tc.tile_pool(name="ib", bufs=4) as ip,
            tc.tile_pool(name="rb", bufs=4) as rp,
            tc.tile_pool(name="st", bufs=4) as sp,
            tc.tile_pool(name="agg", bufs=2, space="PSUM") as aggp,
            tc.tile_pool(name="aux", bufs=1, space="PSUM") as auxp,
            tc.tile_pool(name="poolp", bufs=1, space="PSUM") as poolp,
        ):
            lhs_t = cp.tile([P, nlhs * 64], dt)
            nc.sync.dma_start(out=lhs_t[:], in_=lhsc[:])
            iota_t = cp.tile([P, 64], dt)
            nc.sync.dma_start(out=iota_t[:], in_=iotap[:])
            ident = cp.tile([P, P], dt)
            make_identity(nc, ident[:])
            Wt = {}
            for k, pr in Wp.items():
                shp = list(pr.shape)
                wt_tile = cp.tile(shp, dt, tag=f"w_{k}", name=f"w_{k}")
                Wt[k] = wt_tile
                nc.sync.dma_start(out=Wt[k][:], in_=pr[:])
            invc_t = cp.tile([64, 1], dt)
            nc.sync.dma_start(out=invc_t[:], in_=invc[:])

            # zero the slot buffer once (scatter-add accumulates; layer l
            # subtracts the previous cumulative aggregate instead)
            zt = cp.tile([P, 4096], dt)
            nc.vector.memset(zt[:], 0.0)
            gidx_t = cp.tile([P, gcols], mybir.dt.int16)
            nc.sync.dma_start(out=gidx_t[:], in_=gidx[:])
            sidx_t = cp.tile([P, scols], mybir.dt.int16)
            nc.sync.dma_start(out=sidx_t[:], in_=sidx[:])
            zrows = nreg * WIN
            zm = 4096 // HID  # 64 free-tiles -> 8192 rows per write
            zchunk = P * zm
            for zo in range(0, zrows, zchunk):
                take = min(zchunk, zrows - zo) // P
                nc.sync.dma_start(
                    out=slotbuf[zo:zo + take * P].rearrange("(m p) d -> p m d", p=P),
                    in_=zt[:].rearrange("p (m d) -> p m d", d=HID)[:, :take, :])

            # ---- L0 prep: P0 = x@Wl0 (node-major), S0 = x@Wr0 ----
            for t0 in range(n_store // P):
                lxt = sp.tile([IN_DIM, P], dt, tag="lx")
                nc.sync.dma_start(out=lxt[:], in_=xT[:, t0 * P:(t0 + 1) * P])
                lx = lxt[:]
                psP = auxp.tile([P, HID], dt, tag="pp", space="PSUM")
                nc.tensor.matmul(out=psP[:], lhsT=lx, rhs=Wt["Wl0"][:], start=True, stop=True)
                sb1 = sp.tile([P, HID], dt, tag="pst")
                nc.vector.tensor_copy(out=sb1[:], in_=psP[:])
                nc.sync.dma_start(out=Pshard[t0 * P:(t0 + 1) * P], in_=sb1[:])
                psS = auxp.tile([P, HID], dt, tag="ps", space="PSUM")
                nc.tensor.matmul(out=psS[:], lhsT=lx, rhs=Wt["Wr0"][:], start=True, stop=True)
                sb2 = sp.tile([P, HID], dt, tag="sst")
                nc.vector.tensor_copy(out=sb2[:], in_=psS[:])
                nc.sync.dma_start(out=Scur[t0 * P:(t0 + 1) * P], in_=sb2[:])

            for ell in range(3):
                tc.strict_bb_all_engine_barrier()
                nc.gpsimd.collective_compute(
                    "AllGather", mybir.AluOpType.bypass, replica_groups=rep,
                    ins=[Pshard[:]], outs=[Tglob[:]])
                tc.strict_bb_all_engine_barrier()

                # ---- pass A ----
                ci = 0
                for rr in range(nreg):
                    for ww in range(nwin):
                        wlo = ww * WIN
                        whi = min(wlo + WIN, Ng)
                        for _k in range(int(ncall[rr, ww])):
                            ic0 = ci * (CALLROWS // 16)
                            ic1 = (ci + 1) * (CALLROWS // 16)
                            gt = gp.tile([P, MCALL, HID], dt, tag="g")
                            nc.gpsimd.dma_gather(
                                out_ap=gt[:], in_ap=Tglob[wlo:whi],
                                idxs_ap=gidx_t[:, ic0:ic1],
                                num_idxs=CALLROWS, num_idxs_reg=CALLROWS,
                                elem_size=HID, single_packet=False)
                            nc.gpsimd.dma_scatter_add(
                                out_ap=slotbuf[rr * WIN:(rr + 1) * WIN], in_ap=gt[:],
                                idxs_ap=sidx_t[:, ic0:ic1], num_idxs=CALLROWS,
                                num_idxs_reg=CALLROWS, elem_size=HID,
                                single_packet=False)
                            ci += 1
                tc.strict_bb_all_engine_barrier()

                # ---- pass B ----
                if ell == 2:
                    pool_ps = poolp.tile([64, 64], dt, space="PSUM")
                ti = 0          # global 128-slot tile index
                rtile = None
                rcap = 0
                first_pool = True
                for gi2 in range(0, len(groups), 2):
                    psA = aggp.tile([P, HID], dt, tag="agg", space="PSUM")
                    for half in range(2):
                        g = gi2 + half
                        if g >= len(groups):
                            continue
                        dd, n_sub = groups[g]
                        for s in range(n_sub):
                            if rcap == 0:
                                # slot tile ti starts new read call
                                sl = ti * P
                                rr2 = sl // RUSE
                                off = rr2 * WIN + (sl - rr2 * RUSE)
                                take = min(MCALL, (RUSE - (sl - rr2 * RUSE)) // P,
                                           plan["n_slots"] // P - ti)
                                rtile = rp.tile([P, MCALL, HID], dt, tag="r")
                                nc.sync.dma_start(
                                    out=rtile[:, :take, :],
                                    in_=slotbuf[off:off + take * P].rearrange(
                                        "(m p) d -> p m d", p=P))
                                rcap = take
                                rbase = ti
                            li = lhs_of[(dd, s)]
                            nc.tensor.matmul(
                                out=psA[64 * half:64 * half + 64, :],
                                lhsT=lhs_t[:, li * 64:(li + 1) * 64],
                                rhs=rtile[:, ti - rbase, :],
                                start=(s == 0), stop=(s == n_sub - 1),
                                skip_group_check=True)
                            ti += 1
                            rcap -= 1
                    # combine 128 nodes
                    r0 = gi2 * 64
                    Ssb = sp.tile([P, HID], dt, tag="scomb")
                    nc.sync.dma_start(out=Ssb[:], in_=Scur[r0:r0 + P])
                    t1 = sp.tile([P, HID], dt, tag="t1")
                    if ell > 0:
                        rc = sp.tile([P, HID], dt, tag="rc")
                        nc.sync.dma_start(out=rc[:], in_=rawC[r0:r0 + P])
                        nc.vector.tensor_tensor(out=t1[:], in0=psA[:], in1=rc[:],
                                                op=mybir.AluOpType.subtract)
                    else:
                        nc.vector.tensor_copy(out=t1[:], in_=psA[:])
                    if ell < 2:
                        rcw = sp.tile([P, HID], dt, tag="rcw")
                        nc.vector.tensor_copy(out=rcw[:], in_=psA[:])
                        nc.sync.dma_start(out=rawC[r0:r0 + P], in_=rcw[:])
                    nc.vector.tensor_tensor(out=t1[:], in0=t1[:], in1=Ssb[:],
                                            op=mybir.AluOpType.add)
                    h_t = sp.tile([P, HID], dt, tag="h")
                    bln = Wt[f"bl{ell}"]
                    if ell < 2:
                        nc.vector.tensor_tensor(
                            out=t1[:], in0=t1[:],
                            in1=bln[:],
                            op=mybir.AluOpType.add)
                        nc.scalar.activation(out=h_t[:], in_=t1[:],
                                             func=mybir.ActivationFunctionType.Relu)
                    else:
                        nc.vector.tensor_tensor(
                            out=h_t[:], in0=t1[:],
                            in1=bln[:],
                            op=mybir.AluOpType.add)
                    if ell < 2:
                        # transpose -> h_fm, then P/S for next layer
                        psT = auxp.tile([HID, P], dt, tag="tr", space="PSUM")
                        nc.tensor.transpose(out=psT[:], in_=h_t[:], identity=ident[:])
                        hf = sp.tile([HID, P], dt, tag="hf")
                        nc.vector.tensor_copy(out=hf[:], in_=psT[:])
                        psP = auxp.tile([P, HID], dt, tag="pp", space="PSUM")
                        nc.tensor.matmul(out=psP[:], lhsT=hf[:],
                                         rhs=Wt[f"Wl{ell + 1}"][:], start=True, stop=True)
                        sb1 = sp.tile([P, HID], dt, tag="pst")
                        nc.vector.tensor_copy(out=sb1[:], in_=psP[:])
                        nc.sync.dma_start(out=Pshard[r0:r0 + P], in_=sb1[:])
                        psS = auxp.tile([P, HID], dt, tag="ps", space="PSUM")
                        nc.tensor.matmul(out=psS[:], lhsT=hf[:],
                                         rhs=Wt[f"Wr{ell + 1}"][:], start=True, stop=True)
                        sb2 = sp.tile([P, HID], dt, tag="sst")
                        nc.vector.tensor_copy(out=sb2[:], in_=psS[:])
                        nc.sync.dma_start(out=Snxt[r0:r0 + P], in_=sb2[:])
                    else:
                        bidc = sp.tile([P, 1], dt, tag="bidc")
                        nc.sync.dma_start(out=bidc[:], in_=bidp[r0:r0 + P])
                        oh = sp.tile([P, 64], dt, tag="oh")
                        nc.vector.tensor_tensor(
                            out=oh[:], in0=iota_t[:],
                            in1=bidc[:].to_broadcast([P, 64]),
                            op=mybir.AluOpType.is_equal)
                        nc.tensor.matmul(out=pool_ps[:], lhsT=oh[:], rhs=h_t[:],
                                         start=first_pool, stop=(gi2 + 2 >= len(groups)),
                                         skip_group_check=True)
                        first_pool = False
                if ell < 2:
                    # swap S buffers by copying Snxt -> Scur via DMA? use roles:
                    Scur, Snxt = Snxt, Scur

            # ---- pooled AllReduce + head ----
            pl = sp.tile([64, 64], dt, tag="pl")
            nc.vector.tensor_copy(out=pl[:], in_=pool_ps[:])
            nc.sync.dma_start(out=arin[:], in_=pl[:])
            tc.strict_bb_all_engine_barrier()
            nc.gpsimd.collective_compute(
                "AllReduce", mybir.AluOpType.add, replica_groups=rep,
                ins=[arin[:]], outs=[arout[:]])
            tc.strict_bb_all_engine_barrier()
            pm = sp.tile([64, 64], dt, tag="pm")
            nc.sync.dma_start(out=pm[:], in_=arout[:])
            pmm = sp.tile([64, 64], dt, tag="pmm")
            nc.vector.tensor_scalar_mul(out=pmm[:], in0=pm[:], scalar1=invc_t[:])
            psT2 = auxp.tile([64, 64], dt, tag="tr", space="PSUM")
            nc.tensor.transpose(out=psT2[:], in_=pmm[:], identity=ident[:64, :64])
            pfm = sp.tile([64, 64], dt, tag="pfm")
            nc.vector.tensor_copy(out=pfm[:], in_=psT2[:])
            psO = auxp.tile([64, 1], dt, tag="pp", space="PSUM")
            nc.tensor.matmul(out=psO[:], lhsT=pfm[:], rhs=Wt["Wh"][:], start=True, stop=True)
            ot = sp.tile([64, 1], dt, tag="ot")
            nc.vector.tensor_tensor(out=ot[:], in0=psO[:],
                                    in1=Wt["bh"][:, :1],
                                    op=mybir.AluOpType.add)
            nc.sync.dma_start(out=outp[:], in_=ot[:])

    nc.compile()
    return nc


def kernel(**inputs):
    x = np.asarray(inputs["x"], np.float32)
    plan, per_core, GIDX, SIDX, lhs, gpos = _prep(inputs["edge_index"], inputs["batch"])
    n_store = plan["n_store"]

    batch = np.asarray(inputs["batch"], np.int64)
    counts = np.bincount(batch, minlength=N_GRAPHS).astype(np.float32)
    invc = (1.0 / np.maximum(counts, 1.0)).reshape(64, 1)
    iota = np.broadcast_to(np.arange(64, dtype=np.float32), (P, 64)).copy()

    in_maps = []
    for c in range(NCORE):
        st = per_core["perm"][c]
        xs = np.zeros((n_store, IN_DIM), np.float32)
        real = st >= 0
        xs[real] = x[st[real]]
        m = {
            "xT": np.ascontiguousarray(xs.T),
            "gidx": GIDX[c], "sidx": SIDX[c],
            "lhsc": np.ascontiguousarray(lhs.transpose(1, 0, 2).reshape(P, -1)),
            "bid": per_core["bid"][c].reshape(-1, 1),
            "iota": iota, "invc": invc,
            "Wh": np.asarray(inputs["Wh"], np.float32),
            "bh": np.full((64, 1), float(np.asarray(inputs["bh"]).reshape(-1)[0]), np.float32),
        }
        for ell in range(3):
            m[f"Wl{ell}"] = np.asarray(inputs[f"Wl{ell}"], np.float32)
            m[f"Wr{ell}"] = np.asarray(inputs[f"Wr{ell}"], np.float32)
            m[f"bl{ell}"] = np.broadcast_to(
                np.asarray(inputs[f"bl{ell}"], np.float32), (P, HID)).copy()
        in_maps.append(m)

    nc = _build(plan, GIDX[0].shape[1], SIDX[0].shape[1], lhs.shape[0])

    global LAST_BUILD
    LAST_BUILD = (nc, in_maps)

    from concourse.bass_utils import run_bass_kernel_spmd
    res = run_bass_kernel_spmd(nc, in_maps, list(range(NCORE)))
    return res.results[0]["out"].reshape(64).astype(np.float32)



# revision 5
# speedup vs baseline: 2.8680x; 2.8680x over previous
"""GraphSAGE (3-layer, mean-aggr) on 8 Trainium2 NeuronCores.

Self-contained: host-side graph prep (numpy) + Bass kernel build + SPMD run.

Architecture per layer:
  P = h @ Wl per shard -> AllGather table T [Ng, 64]
  pass A: per (slot-region r, src-window w): dma_gather rows of T (int16 idx,
          windowed) -> dma_scatter_add into unique dst-grouped slots of a
          slot buffer (int16, windowed, collision-free)
  pass B: sequential reads of slot stream -> constant block-band matmuls on PE
          (aggregation, 1/deg folded into the constants; 64-node groups, PSUM
          partition starts 0/64) -> node-major M
  combine: h' = relu(M + S + b), S = h @ Wr kept from previous layer
  tail: per-128-node-tile PE transpose -> h_fm -> P/S for next layer
  L2: one-hot pooling matmul accumulated in PSUM -> AllReduce -> head.
"""
import numpy as np

import concourse.bass as bass
import concourse.mybir as mybir
import concourse.tile as tile
from concourse import bacc
from concourse.masks import make_identity

N_NODES = 250000
N_EDGES = 2000000
N_GRAPHS = 64
IN_DIM = 16
HID = 64
NCORE = 8
P = 128
WIN = 32768          # int16 index window (rows)
RUSE = 30720         # usable slots per region
RSCR = 2048          # scratch slots per region
MCALL = 16           # free-tiles per gather/scatter call (2048 rows)
CALLROWS = P * MCALL


def _wrap_idx(flat):
    """[n] -> [128, n//16] int16: stream i at [i%16, i//16], replicated x8."""
    n = flat.shape[0]
    base = flat.reshape(n // 16, 16).T.astype(np.int16)
    t = np.zeros((P, n // 16), np.int16)
    for g in range(8):
        t[g * 16:(g + 1) * 16] = base
    return t


def _prep(edge_index, batch):
    src_g = np.asarray(edge_index[0], np.int64)
    dst_g = np.asarray(edge_index[1], np.int64)
    batch = np.asarray(batch, np.int64)

    base = N_NODES // NCORE  # 31250
    core_of = np.minimum(dst_g // base, NCORE - 1)
    deg = np.bincount(dst_g, minlength=N_NODES)

    # per-core: nodes [c*base, (c+1)*base); class grouping by in-degree
    cores = []
    dmax = int(deg.max())
    census = np.zeros((NCORE, dmax + 1), np.int64)
    for c in range(NCORE):
        lo, hi = c * base, (c + 1) * base if c < NCORE - 1 else N_NODES
        nodes = np.arange(lo, hi)
        d = deg[lo:hi]
        order = np.argsort(d, kind="stable")
        cores.append((nodes[order], d[order]))
        for dd in range(dmax + 1):
            census[c, dd] = int((d == dd).sum())
    # equalize: per-class node count = max over cores, padded to multiple of 64
    ncls = np.zeros(dmax + 1, np.int64)
    for dd in range(dmax + 1):
        m = int(census[:, dd].max())
        if m > 0:
            ncls[dd] = ((m + 63) // 64) * 64
    n_store = int(ncls.sum())
    n_store = ((n_store + 127) // 128) * 128
    ncls[0] += n_store - int(ncls.sum())  # absorb rounding into class 0

    # storage layout: classes ascending d; within class: real nodes then fakes
    # slot stream: class 0 -> 1 slot/node (zero lhsT); class d -> d slots/node
    # groups of 64 nodes; per group ceil(64*max(d,1)/128) tiles of 128 slots
    cls_start = np.zeros(dmax + 2, np.int64)
    for dd in range(dmax + 1):
        cls_start[dd + 1] = cls_start[dd] + ncls[dd]

    # tile plan (shared across cores): list of (d, sub) per 128-slot tile and
    # group plan: per 64-group: (d, n_sub); slot position of (class,node,j)
    tiles = []          # (d, sub_idx)
    groups = []         # (d, n_sub)
    slot_of_group = []  # slot index of group start
    scur = 0
    for dd in range(dmax + 1):
        de = max(dd, 1)
        n_sub = (64 * de + 127) // 128
        for g in range(int(ncls[dd]) // 64):
            groups.append((dd, n_sub))
            slot_of_group.append(scur)
            for s in range(n_sub):
                tiles.append((dd, s))
            scur += n_sub * 128
    n_slots = scur
    nreg = (n_slots + RUSE - 1) // RUSE

    # per-core data
    per_core = {"gidx": [], "sidx": [], "perm": [], "bid": []}
    # slot id for (group, node_in_group j, edge k): group_slot + j*de + k
    # (pad slots j*de+k beyond group's 64*de... none: slots per group n_sub*128
    #  >= 64*de; layout: node j edges at [j*de, (j+1)*de) contiguous; pad tail)
    callplan = None
    for c in range(NCORE):
        nodes_sorted, d_sorted = cores[c]
        # storage position of each real node
        storage = np.full(n_store, -1, np.int64)   # storage -> orig node
        pos = {}
        cur = 0
        for dd in range(dmax + 1):
            sel = d_sorted == dd
            nn = nodes_sorted[sel]
            storage[cls_start[dd]:cls_start[dd] + len(nn)] = nn
            cur = 0
            for i, v in enumerate(nn):
                pos[v] = cls_start[dd] + i
        per_core["perm"].append(storage)
        bid = np.full(n_store, -1.0, np.float32)
        real = storage >= 0
        bid[real] = batch[storage[real]].astype(np.float32)
        per_core["bid"].append(bid)

        # edges of this core -> slots
        em = core_of == c
        es, ed = src_g[em], dst_g[em]
        # slot for edge: node storage pos -> group, j
        p_arr = np.array([pos[v] for v in ed], np.int64)
        dd_arr = deg[ed]
        grp = p_arr // 64
        j = p_arr % 64
        de_arr = np.maximum(dd_arr, 1)
        # rank of edge within its node (stable order)
        order = np.argsort(p_arr, kind="stable")
        es, ed, p_arr, grp, j, de_arr = (a[order] for a in (es, ed, p_arr, grp, j, de_arr))
        k = np.zeros(len(es), np.int64)
        if len(p_arr):
            brk = np.r_[True, p_arr[1:] != p_arr[:-1]]
            idx0 = np.flatnonzero(brk)
            k = np.arange(len(p_arr)) - np.repeat(idx0, np.diff(np.r_[idx0, len(p_arr)]))
        gstart = np.array(slot_of_group, np.int64)
        slot = gstart[grp] + j * de_arr + k
        # class-0 nodes: add one zero-slot edge each (src -> global row 0)
        z_pos = np.arange(cls_start[0], cls_start[1])
        z_grp = z_pos // 64
        z_slot = gstart[z_grp] + (z_pos % 64)
        # table row of src: core_of(src)*n_store + storage pos on that core
        # (need pos map of OTHER cores -> compute later; placeholder store orig)
        per_core["gidx"].append((es, slot, z_slot))

    # global storage position of every original node
    gpos = np.zeros(N_NODES, np.int64)
    for c in range(NCORE):
        st = per_core["perm"][c]
        real = st >= 0
        gpos[st[real]] = c * n_store + np.flatnonzero(real)

    # build call plans (identical structure across cores)
    # region r: slots [r*32768+0, ... r*32768+30720) hold stream chunk
    # stream slot s -> buffer address r*32768 + (s - r*RUSE)
    nwin = (NCORE * n_store + WIN - 1) // WIN
    cnt = np.zeros((NCORE, nreg, nwin), np.int64)
    edata = []
    for c in range(NCORE):
        es, slot, z_slot = per_core["gidx"][c]
        row = gpos[es]
        w = row // WIN
        r = slot // RUSE
        zrow = np.zeros(len(z_slot), np.int64)
        zw = zrow // WIN
        zr = z_slot // RUSE
        row = np.r_[row, zrow]
        w = np.r_[w, zw]
        slot_all = np.r_[slot, z_slot]
        r = np.r_[r, zr]
        key = r * nwin + w
        bc = np.bincount(key, minlength=nreg * nwin)
        cnt[c] = bc.reshape(nreg, nwin)
        order2 = np.argsort(key, kind="stable")
        edata.append((row[order2], w[order2], slot_all[order2], r[order2],
                      np.r_[0, np.cumsum(bc)]))
    ncall = np.zeros((nreg, nwin), np.int64)
    for rr in range(nreg):
        for ww in range(nwin):
            m = int(cnt[:, rr, ww].max())
            ncall[rr, ww] = (m + CALLROWS - 1) // CALLROWS

    GIDX, SIDX = [], []
    for c in range(NCORE):
        row, w, slot_all, r, csum = edata[c]
        gi_cols, si_cols = [], []
        for rr in range(nreg):
            for ww in range(nwin):
                need = int(ncall[rr, ww] * CALLROWS)
                a, b = csum[rr * nwin + ww], csum[rr * nwin + ww + 1]
                rws = row[a:b] - ww * WIN
                sls = slot_all[a:b] - rr * RUSE
                npad = need - len(rws)
                scr = RUSE + (np.arange(npad) % RSCR)
                rws = np.r_[rws, np.zeros(npad, np.int64)]
                sls = np.r_[sls, scr]
                for kk in range(int(ncall[rr, ww])):
                    gi_cols.append(_wrap_idx(rws[kk * CALLROWS:(kk + 1) * CALLROWS]))
                    si_cols.append(_wrap_idx(sls[kk * CALLROWS:(kk + 1) * CALLROWS]))
        GIDX.append(np.concatenate(gi_cols, 1))
        SIDX.append(np.concatenate(si_cols, 1))

    # lhsT constants: per (d, sub): [128, 64] with lhsT[s, jj] = 1/d if slot
    # sub*128+s belongs to node jj (slots j*de..j*de+de), 0 for class 0
    lhs = []
    lhs_of = {}
    for dd in range(dmax + 1):
        de = max(dd, 1)
        n_sub = (64 * de + 127) // 128
        for s in range(n_sub):
            m = np.zeros((P, 64), np.float32)
            if dd > 0:
                for srow in range(P):
                    gslot = s * 128 + srow
                    jj = gslot // de
                    if jj < 64:
                        m[srow, jj] = 1.0 / dd
            lhs_of[(dd, s)] = len(lhs)
            lhs.append(m)
    lhs = np.stack(lhs) if lhs else np.zeros((1, P, 64), np.float32)

    plan = dict(n_store=n_store, n_slots=n_slots, nreg=nreg, nwin=nwin,
                ncall=ncall, tiles=tiles, groups=groups, lhs_of=lhs_of,
                dmax=dmax)
    return plan, per_core, GIDX, SIDX, lhs, gpos


def _build(plan, gcols, scols, nlhs):
    nc = bacc.Bacc(None, target_bir_lowering=False, num_swdge_queues=4)
    dt = mybir.dt.float32
    n_store = plan["n_store"]
    nreg, nwin, ncall = plan["nreg"], plan["nwin"], plan["ncall"]
    Ng = NCORE * n_store

    xT = nc.declare_dram_parameter("xT", [IN_DIM, n_store], dt, isOutput=False)
    gidx = nc.declare_dram_parameter("gidx", [P, gcols], mybir.dt.int16, isOutput=False)
    sidx = nc.declare_dram_parameter("sidx", [P, scols], mybir.dt.int16, isOutput=False)
    lhsc = nc.declare_dram_parameter("lhsc", [P, nlhs * 64], dt, isOutput=False)
    bidp = nc.declare_dram_parameter("bid", [n_store, 1], dt, isOutput=False)
    iotap = nc.declare_dram_parameter("iota", [P, 64], dt, isOutput=False)
    invc = nc.declare_dram_parameter("invc", [64, 1], dt, isOutput=False)
    Wp = {}
    for ell in range(3):
        kd = IN_DIM if ell == 0 else HID
        Wp[f"Wl{ell}"] = nc.declare_dram_parameter(f"Wl{ell}", [kd, HID], dt, isOutput=False)
        Wp[f"Wr{ell}"] = nc.declare_dram_parameter(f"Wr{ell}", [kd, HID], dt, isOutput=False)
        Wp[f"bl{ell}"] = nc.declare_dram_parameter(f"bl{ell}", [P, HID], dt, isOutput=False)
    Wp["Wh"] = nc.declare_dram_parameter("Wh", [HID, 1], dt, isOutput=False)
    Wp["bh"] = nc.declare_dram_parameter("bh", [64, 1], dt, isOutput=False)
    outp = nc.declare_dram_parameter("out", [64, 1], dt, isOutput=True)

    Pshard = nc.dram_tensor("Pshard", [n_store, HID], dt)
    Tglob = nc.dram_tensor("Tglob", [Ng, HID], dt, addr_space="Shared")
    slotbuf = nc.dram_tensor("slotbuf", [nreg * WIN, HID], dt)
    Scur = nc.dram_tensor("Scur", [n_store, HID], dt)
    Snxt = nc.dram_tensor("Snxt", [n_store, HID], dt)
    rawC = nc.dram_tensor("rawC", [n_store, HID], dt)
    arin = nc.dram_tensor("arin", [64, 64], dt)
    arout = nc.dram_tensor("arout", [64, 64], dt, addr_space="Shared")

    rep = [list(range(NCORE))]
    tiles, groups, lhs_of = plan["tiles"], plan["groups"], plan["lhs_of"]

    with tile.TileContext(nc) as tc:
        with (
            tc.tile_pool(name="const", bufs=1) as cp,
            tc.tile_pool(name="gb", bufs=8) as gp,
            tc.tile_pool(name="ib", bufs=4) as ip,
            tc.tile_pool(name="rb", bufs=4) as rp,
            tc.tile_pool(name="st", bufs=4) as sp,
            tc.tile_pool(name="agg", bufs=2, space="PSUM") as aggp,
            tc.tile_pool(name="aux", bufs=1, space="PSUM") as auxp,
            tc.tile_pool(name="poolp", bufs=1, space="PSUM") as poolp,
        ):
            lhs_t = cp.tile([P, nlhs * 64], dt)
            nc.sync.dma_start(out=lhs_t[:], in_=lhsc[:])
            iota_t = cp.tile([P, 64], dt)
            nc.sync.dma_start(out=iota_t[:], in_=iotap[:])
            ident = cp.tile([P, P], dt)
            make_identity(nc, ident[:])
            Wt = {}
            for k, pr in Wp.items():
                shp = list(pr.shape)
                wt_tile = cp.tile(shp, dt, tag=f"w_{k}", name=f"w_{k}")
                Wt[k] = wt_tile
                nc.sync.dma_start(out=Wt[k][:], in_=pr[:])
            invc_t = cp.tile([64, 1], dt)
            nc.sync.dma_start(out=invc_t[:], in_=invc[:])

            # zero the slot buffer once (scatter-add accumulates; layer l
            # subtracts the previous cumulative aggregate instead)
            zt = cp.tile([P, 4096], dt)
            nc.vector.memset(zt[:], 0.0)
            gidx_t = cp.tile([P, gcols], mybir.dt.int16)
            nc.sync.dma_start(out=gidx_t[:], in_=gidx[:])
            sidx_t = cp.tile([P, scols], mybir.dt.int16)
            nc.sync.dma_start(out=sidx_t[:], in_=sidx[:])
            zrows = nreg * WIN
            zm = 4096 // HID  # 64 free-tiles -> 8192 rows per write
            zchunk = P * zm
            for zo in range(0, zrows, zchunk):
                take = min(zchunk, zrows - zo) // P
                nc.sync.dma_start(
                    out=slotbuf[zo:zo + take * P].rearrange("(m p) d -> p m d", p=P),
                    in_=zt[:].rearrange("p (m d) -> p m d", d=HID)[:, :take, :])

            # ---- L0 prep: P0 = x@Wl0 (node-major), S0 = x@Wr0 ----
            for t0 in range(n_store // P):
                lxt = sp.tile([IN_DIM, P], dt, tag="lx")
                nc.sync.dma_start(out=lxt[:], in_=xT[:, t0 * P:(t0 + 1) * P])
                lx = lxt[:]
                psP = auxp.tile([P, HID], dt, tag="pp", space="PSUM")
                nc.tensor.matmul(out=psP[:], lhsT=lx, rhs=Wt["Wl0"][:], start=True, stop=True)
                sb1 = sp.tile([P, HID], dt, tag="pst")
                nc.vector.tensor_copy(out=sb1[:], in_=psP[:])
                nc.sync.dma_start(out=Pshard[t0 * P:(t0 + 1) * P], in_=sb1[:])
                psS = auxp.tile([P, HID], dt, tag="ps", space="PSUM")
                nc.tensor.matmul(out=psS[:], lhsT=lx, rhs=Wt["Wr0"][:], start=True, stop=True)
                sb2 = sp.tile([P, HID], dt, tag="sst")
                nc.vector.tensor_copy(out=sb2[:], in_=psS[:])
                nc.sync.dma_start(out=Scur[t0 * P:(t0 + 1) * P], in_=sb2[:])

            for ell in range(3):
                tc.strict_bb_all_engine_barrier()
                nc.gpsimd.collective_compute(
                    "AllGather", mybir.AluOpType.bypass, replica_groups=rep,
                    ins=[Pshard[:]], outs=[Tglob[:]])
                tc.strict_bb_all_engine_barrier()

                # ---- pass A ----
                ci = 0
                for rr in range(nreg):
                    for ww in range(nwin):
                        wlo = ww * WIN
                        whi = min(wlo + WIN, Ng)
                        for _k in range(int(ncall[rr, ww])):
                            ic0 = ci * (CALLROWS // 16)
                            ic1 = (ci + 1) * (CALLROWS // 16)
                            gt = gp.tile([P, MCALL, HID], dt, tag="g")
                            nc.gpsimd.dma_gather(
                                out_ap=gt[:], in_ap=Tglob[wlo:whi],
                                idxs_ap=gidx_t[:, ic0:ic1],
                                num_idxs=CALLROWS, num_idxs_reg=CALLROWS,
                                elem_size=HID, single_packet=False,
                                queue_num=ci % 2)
                            nc.gpsimd.dma_scatter_add(
                                out_ap=slotbuf[rr * WIN:(rr + 1) * WIN], in_ap=gt[:],
                                idxs_ap=sidx_t[:, ic0:ic1], num_idxs=CALLROWS,
                                num_idxs_reg=CALLROWS, elem_size=HID,
                                single_packet=False, queue_num=2 + ci % 2)
                            ci += 1
                tc.strict_bb_all_engine_barrier()

                # ---- pass B ----
                if ell == 2:
                    pool_ps = poolp.tile([64, 64], dt, space="PSUM")
                ti = 0          # global 128-slot tile index
                rtile = None
                rcap = 0
                first_pool = True
                for gi2 in range(0, len(groups), 2):
                    psA = aggp.tile([P, HID], dt, tag="agg", space="PSUM")
                    for half in range(2):
                        g = gi2 + half
                        if g >= len(groups):
                            continue
                        dd, n_sub = groups[g]
                        for s in range(n_sub):
                            if rcap == 0:
                                # slot tile ti starts new read call
                                sl = ti * P
                                rr2 = sl // RUSE
                                off = rr2 * WIN + (sl - rr2 * RUSE)
                                take = min(MCALL, (RUSE - (sl - rr2 * RUSE)) // P,
                                           plan["n_slots"] // P - ti)
                                rtile = rp.tile([P, MCALL, HID], dt, tag="r")
                                nc.sync.dma_start(
                                    out=rtile[:, :take, :],
                                    in_=slotbuf[off:off + take * P].rearrange(
                                        "(m p) d -> p m d", p=P))
                                rcap = take
                                rbase = ti
                            li = lhs_of[(dd, s)]
                            nc.tensor.matmul(
                                out=psA[64 * half:64 * half + 64, :],
                                lhsT=lhs_t[:, li * 64:(li + 1) * 64],
                                rhs=rtile[:, ti - rbase, :],
                                start=(s == 0), stop=(s == n_sub - 1),
                                skip_group_check=True)
                            ti += 1
                            rcap -= 1
                    # combine 128 nodes
                    r0 = gi2 * 64
                    Ssb = sp.tile([P, HID], dt, tag="scomb")
                    nc.sync.dma_start(out=Ssb[:], in_=Scur[r0:r0 + P])
                    t1 = sp.tile([P, HID], dt, tag="t1")
                    if ell > 0:
                        rc = sp.tile([P, HID], dt, tag="rc")
                        nc.sync.dma_start(out=rc[:], in_=rawC[r0:r0 + P])
                        nc.vector.tensor_tensor(out=t1[:], in0=psA[:], in1=rc[:],
                                                op=mybir.AluOpType.subtract)
                    else:
                        nc.vector.tensor_copy(out=t1[:], in_=psA[:])
                    if ell < 2:
                        rcw = sp.tile([P, HID], dt, tag="rcw")
                        nc.vector.tensor_copy(out=rcw[:], in_=psA[:])
                        nc.sync.dma_start(out=rawC[r0:r0 + P], in_=rcw[:])
                    nc.vector.tensor_tensor(out=t1[:], in0=t1[:], in1=Ssb[:],
                                            op=mybir.AluOpType.add)
                    h_t = sp.tile([P, HID], dt, tag="h")
                    bln = Wt[f"bl{ell}"]
                    if ell < 2:
                        nc.vector.tensor_tensor(
                            out=t1[:], in0=t1[:],
                            in1=bln[:],
                            op=mybir.AluOpType.add)
                        nc.scalar.activation(out=h_t[:], in_=t1[:],
                                             func=mybir.ActivationFunctionType.Relu)
                    else:
                        nc.vector.tensor_tensor(
                            out=h_t[:], in0=t1[:],
                            in1=bln[:],
                            op=mybir.AluOpType.add)
                    if ell < 2:
                        # transpose -> h_fm, then P/S for next layer
                        psT = auxp.tile([HID, P], dt, tag="tr", space="PSUM")
                        nc.tensor.transpose(out=psT[:], in_=h_t[:], identity=ident[:])
                        hf = sp.tile([HID, P], dt, tag="hf")
                        nc.vector.tensor_copy(out=hf[:], in_=psT[:])
                        psP = auxp.tile([P, HID], dt, tag="pp", space="PSUM")
                        nc.tensor.matmul(out=psP[:], lhsT=hf[:],
                                         rhs=Wt[f"Wl{ell + 1}"][:], start=True, stop=True)
                        sb1 = sp.tile([P, HID], dt, tag="pst")
                        nc.vector.tensor_copy(out=sb1[:], in_=psP[:])
                        nc.sync.dma_start(out=Pshard[r0:r0 + P], in_=sb1[:])
                        psS = auxp.tile([P, HID], dt, tag="ps", space="PSUM")
                        nc.tensor.matmul(out=psS[:], lhsT=hf[:],
                                         rhs=Wt[f"Wr{ell + 1}"][:], start=True, stop=True)
                        sb2 = sp.tile([P, HID], dt, tag="sst")
                        nc.vector.tensor_copy(out=sb2[:], in_=psS[:])
                        nc.sync.dma_start(out=Snxt[r0:r0 + P], in_=sb2[:])
                    else:
                        bidc = sp.tile([P, 1], dt, tag="bidc")
                        nc.sync.dma_start(out=bidc[:], in_=bidp[r0:r0 + P])
                        oh = sp.tile([P, 64], dt, tag="oh")
                        nc.vector.tensor_tensor(
                            out=oh[:], in0=iota_t[:],
                            in1=bidc[:].to_broadcast([P, 64]),
                            op=mybir.AluOpType.is_equal)
                        nc.tensor.matmul(out=pool_ps[:], lhsT=oh[:], rhs=h_t[:],
                                         start=first_pool, stop=(gi2 + 2 >= len(groups)),
                                         skip_group_check=True)
                        first_pool = False
                if ell < 2:
                    # swap S buffers by copying Snxt -> Scur via DMA? use roles:
                    Scur, Snxt = Snxt, Scur

            # ---- pooled AllReduce + head ----
            pl = sp.tile([64, 64], dt, tag="pl")
            nc.vector.tensor_copy(out=pl[:], in_=pool_ps[:])
            nc.sync.dma_start(out=arin[:], in_=pl[:])
            tc.strict_bb_all_engine_barrier()
            nc.gpsimd.collective_compute(
                "AllReduce", mybir.AluOpType.add, replica_groups=rep,
                ins=[arin[:]], outs=[arout[:]])
            tc.strict_bb_all_engine_barrier()
            pm = sp.tile([64, 64], dt, tag="pm")
            nc.sync.dma_start(out=pm[:], in_=arout[:])
            pmm = sp.tile([64, 64], dt, tag="pmm")
            nc.vector.tensor_scalar_mul(out=pmm[:], in0=pm[:], scalar1=invc_t[:])
            psT2 = auxp.tile([64, 64], dt, tag="tr", space="PSUM")
            nc.tensor.transpose(out=psT2[:], in_=pmm[:], identity=ident[:64, :64])
            pfm = sp.tile([64, 64], dt, tag="pfm")
            nc.vector.tensor_copy(out=pfm[:], in_=psT2[:])
            psO = auxp.tile([64, 1], dt, tag="pp", space="PSUM")
            nc.tensor.matmul(out=psO[:], lhsT=pfm[:], rhs=Wt["Wh"][:], start=True, stop=True)
            ot = sp.tile([64, 1], dt, tag="ot")
            nc.vector.tensor_tensor(out=ot[:], in0=psO[:],
                                    in1=Wt["bh"][:, :1],
                                    op=mybir.AluOpType.add)
            nc.sync.dma_start(out=outp[:], in_=ot[:])

    nc.compile()
    return nc


def kernel(**inputs):
    x = np.asarray(inputs["x"], np.float32)
    plan, per_core, GIDX, SIDX, lhs, gpos = _prep(inputs["edge_index"], inputs["batch"])
    n_store = plan["n_store"]

    batch = np.asarray(inputs["batch"], np.int64)
    counts = np.bincount(batch, minlength=N_GRAPHS).astype(np.float32)
    invc = (1.0 / np.maximum(counts, 1.0)).reshape(64, 1)
    iota = np.broadcast_to(np.arange(64, dtype=np.float32), (P, 64)).copy()

    in_maps = []
    for c in range(NCORE):
        st = per_core["perm"][c]
        xs = np.zeros((n_store, IN_DIM), np.float32)
        real = st >= 0
        xs[real] = x[st[real]]
        m = {
            "xT": np.ascontiguousarray(xs.T),
            "gidx": GIDX[c], "sidx": SIDX[c],
            "lhsc": np.ascontiguousarray(lhs.transpose(1, 0, 2).reshape(P, -1)),
            "bid": per_core["bid"][c].reshape(-1, 1),
            "iota": iota, "invc": invc,
            "Wh": np.asarray(inputs["Wh"], np.float32),
            "bh": np.full((64, 1), float(np.asarray(inputs["bh"]).reshape(-1)[0]), np.float32),
        }
        for ell in range(3):
            m[f"Wl{ell}"] = np.asarray(inputs[f"Wl{ell}"], np.float32)
            m[f"Wr{ell}"] = np.asarray(inputs[f"Wr{ell}"], np.float32)
            m[f"bl{ell}"] = np.broadcast_to(
                np.asarray(inputs[f"bl{ell}"], np.float32), (P, HID)).copy()
        in_maps.append(m)

    nc = _build(plan, GIDX[0].shape[1], SIDX[0].shape[1], lhs.shape[0])

    global LAST_BUILD
    LAST_BUILD = (nc, in_maps)

    from concourse.bass_utils import run_bass_kernel_spmd
    res = run_bass_kernel_spmd(nc, in_maps, list(range(NCORE)))
    return res.results[0]["out"].reshape(64).astype(np.float32)

